# revision 13
# baseline (speedup 1.0000x reference)
"""Fused causal-attention block (QKV proj + causal softmax attention + out proj
+ residual + LayerNorm) on 8 Trainium2 NeuronCores.

Sharding: core c -> batch b = c//4, head-group r = c%4 (heads 4r..4r+3,
d' columns 256r..256r+256).  All matmul operands are bf16 (fp32 PSUM
accumulation), which halves DMA traffic and runs the PE at 1 cycle/row.

Single fused pipeline per core, streaming over the four 512-wide n-chunks:
  chunk nt: load xT chunk -> Q/K/V projections for the chunk
            -> causal attention for q-tile qt=nt (flash-style, no max
               subtraction; denominators via an all-ones column in V)
            -> partial output projection (only this core's 256 d' columns
               of Wo) -> stage partial [512,1024] bf16 to DRAM
            -> ReduceScatter(add) over the batch's 4 cores: rank r gets
               rows [128r,128r+128) of the q-tile summed.
Each RS overlaps later chunks' compute; only the last one is exposed.
A final phase reads back the four [128,1024] RS shards, adds the fp32
residual rows, and runs LayerNorm + store.  Host reassembles 8 cores x
4 q-tiles x 128 rows.

Causal masking on diagonal 128x128 blocks: exp(scores) multiplied by an
upper-triangular 0/1 bf16 matrix on the DVE.  The two heads of a pair
compute their K=64 score matmuls at PE row bases 0/64 (disjoint row
groups -> concurrent) into one shared [128,2,512] PSUM tile; one strided
ACT exp call covers both heads.
"""

import numpy as np

B, N, D = 2, 2048, 1024
H, DH = 16, 64
NCORES = 8
HPC = 4          # heads per core
DP = HPC * DH    # 256 d' columns per core
NQ = N // 4      # 512 rows per q-tile (and per core)
LN_EPS = 1e-5
GROUPS = [[0, 1, 2, 3], [4, 5, 6, 7]]

_CACHE = {}


def _build(flags):
    """Build+compile the Bacc program. flags = (has_qkv_bias, has_gamma, has_beta)."""
    import concourse.bass as bass
    import concourse.bacc as bacc
    import concourse.tile as tile
    from concourse import mybir
    from contextlib import ExitStack

    has_qkv_bias, has_gamma, has_beta = flags
    f32 = mybir.dt.float32
    bf16 = mybir.dt.bfloat16
    AF = mybir.ActivationFunctionType
    ALU = mybir.AluOpType

    nc = bacc.Bacc(
        trn_type="TRN2",
        target_bir_lowering=False,
        debug=False,
        num_devices=NCORES,
    )

    xT = nc.dram_tensor("xT", [D, N], bf16, kind="ExternalInput").ap()
    xres = nc.dram_tensor("xres", [NQ, D], f32, kind="ExternalInput").ap()
    wqT = nc.dram_tensor("wqT", [D, DP], bf16, kind="ExternalInput").ap()
    wkT = nc.dram_tensor("wkT", [D, DP], bf16, kind="ExternalInput").ap()
    wvT = nc.dram_tensor("wvT", [D, DP], bf16, kind="ExternalInput").ap()
    woTm = nc.dram_tensor("woTm", [DP, D], bf16, kind="ExternalInput").ap()
    out = nc.dram_tensor("out", [NQ, D], f32, kind="ExternalOutput").ap()
    if has_qkv_bias:
        bqkv = nc.dram_tensor("bqkv", [1, 3, DP], bf16, kind="ExternalInput").ap()
    if has_gamma:
        gamma_d = nc.dram_tensor("gamma", [D], f32, kind="ExternalInput").ap()
    if has_beta:
        beta_d = nc.dram_tensor("beta", [D], f32, kind="ExternalInput").ap()

    # multiplicative causal mask for diagonal blocks: keep k <= q
    # (partition p = k offset, free c = q offset)
    tri_np = np.triu(np.ones((128, 128), np.float32))
    tri_d = nc.inline_tensor(tri_np, name="tri_const").ap()

    with tile.TileContext(nc) as tc, ExitStack() as ctx, \
            nc.allow_low_precision(reason="bf16 operands, fp32 accumulation"):
        singles = ctx.enter_context(tc.tile_pool(name="singles", bufs=1))
        qkv_pool = ctx.enter_context(tc.tile_pool(name="qkv", bufs=1))

        # weights, striped k-on-partitions (one per queue, then x chunks)
        wq_sb = singles.tile([128, 8, DP], bf16, tag="wq")
        wk_sb = singles.tile([128, 8, DP], bf16, tag="wk")
        wv_sb = singles.tile([128, 8, DP], bf16, tag="wv")
        nc.sync.dma_start(wq_sb, wqT.rearrange("(ko p) m -> p ko m", p=128))
        nc.scalar.dma_start(wk_sb, wkT.rearrange("(ko p) m -> p ko m", p=128))
        nc.gpsimd.dma_start(wv_sb, wvT.rearrange("(ko p) m -> p ko m", p=128))

        # prefetch the whole x^T, chunk-major so chunk 0 lands first;
        # every later phase's DMA sits behind these on the three queues.
        xT_sb = qkv_pool.tile([128, 8, N], bf16, tag="xT")
        xT_r = xT.rearrange("(ko p) n -> ko p n", p=128)
        dma_engs = [nc.sync, nc.scalar, nc.gpsimd]
        for nt in range(4):
            for ko in range(8):
                dma_engs[ko % 3].dma_start(
                    xT_sb[:, ko, 512 * nt:512 * (nt + 1)],
                    xT_r[ko][:, 512 * nt:512 * (nt + 1)])

        wo_sb = singles.tile([128, 2, D], bf16, tag="wo")
        nc.scalar.dma_start(wo_sb, woTm.rearrange("(t p) m -> p t m", p=128))

        tri_f = singles.tile([128, 128], f32, tag="tri_f")
        nc.sync.dma_start(tri_f, tri_d)
        tri_sb = singles.tile([128, 128], bf16, tag="tri")
        nc.vector.tensor_copy(out=tri_sb, in_=tri_f)

        ones_f32 = singles.tile([128, 64], f32, tag="ones_f32")
        nc.vector.memset(ones_f32, 1.0)
        ones64 = singles.tile([1, 64], bf16, tag="ones64")
        nc.vector.tensor_copy(out=ones64, in_=ones_f32[0:1, :])
        eps_sb = singles.tile([128, 1], f32, tag="eps")
        nc.vector.memset(eps_sb, LN_EPS)
        if has_qkv_bias:
            o512f = singles.tile([1, 512], f32, tag="o512f")
            nc.vector.memset(o512f, 1.0)
            ones512 = singles.tile([1, 512], bf16, tag="ones512")
            nc.vector.tensor_copy(out=ones512, in_=o512f)
            bqkv_sb = singles.tile([1, 3, DP], bf16, tag="bqkv")
            nc.sync.dma_start(bqkv_sb, bqkv)
        if has_gamma:
            gamma_sb = singles.tile([128, D], f32, tag="gamma")
            nc.sync.dma_start(
                gamma_sb,
                bass.AP(tensor=gamma_d.tensor, offset=gamma_d.offset,
                        ap=[[0, 128]] + gamma_d.ap),
            )
        if has_beta:
            beta_sb = singles.tile([128, D], f32, tag="beta")
            nc.sync.dma_start(
                beta_sb,
                bass.AP(tensor=beta_d.tensor, offset=beta_d.offset,
                        ap=[[0, 128]] + beta_d.ap),
            )

        # persistent activations
        qT_sb = qkv_pool.tile([128, 2, N], bf16, tag="qT")   # Q^T [d'(256), n]
        kT_sb = qkv_pool.tile([128, 2, N], bf16, tag="kT")   # K^T [d'(256), n]
        v_sb = qkv_pool.tile([128, 16, HPC, DH + 1], bf16, tag="v")  # V + ones
        ctx_sb = qkv_pool.tile([128, 2, N], bf16, tag="ctxT")  # normalized ctx^T
        nc.vector.tensor_copy(
            out=v_sb[:, :, :, DH:DH + 1],
            in_=ones_f32.rearrange("p (a b c) -> p a b c", a=16, b=4))

        # residual rows (pure-input DMAs; queued behind the x prefetch)
        xres_sb = singles.tile([128, 4, D], f32, tag="xres")
        for t in range(4):
            [nc.sync, nc.scalar, nc.gpsimd][t % 3].dma_start(
                xres_sb[:, t], xres[128 * t:128 * (t + 1)])

        dram_pool = ctx.enter_context(tc.tile_pool(name="dram", bufs=1,
                                                   space="DRAM"))
        part_dram = [dram_pool.tile([NQ, D], bf16, tag=f"part{qt}",
                                    name=f"part{qt}")
                     for qt in range(4)]
        rsout_dram = [dram_pool.tile([128, D], bf16, tag=f"rsout{qt}",
                                     name=f"rsout{qt}")
                      for qt in range(4)]

        # PSUM pools: mm512 (QKV + out-proj) 2 banks, sp 4 banks, cps 2 banks
        mm512 = ctx.enter_context(tc.tile_pool(name="mm512", bufs=2,
                                               space="PSUM"))
        sps_pool = ctx.enter_context(tc.tile_pool(name="sps", bufs=2,
                                                  space="PSUM"))
        cps_pool = ctx.enter_context(tc.tile_pool(name="cps", bufs=1,
                                                  space="PSUM"))
        es_pool = ctx.enter_context(tc.tile_pool(name="es", bufs=6))
        nrm_pool = ctx.enter_context(tc.tile_pool(name="nrm", bufs=4))
        part_pool = ctx.enter_context(tc.tile_pool(name="part", bufs=2))

        def emit_recip(qt, hp, cps):
            recs = []
            for hi, h in enumerate((2 * hp, 2 * hp + 1)):
                rec = nrm_pool.tile([1, 512], bf16, tag="rec",
                                    name=f"rec_{qt}_{h}")
                nc.vector.reciprocal(out=rec, in_=cps[64:65, hi, :])
                recs.append(rec)
            return recs

        def emit_normalize(qt, hp, cps, recs):
            for hi, h in enumerate((2 * hp, 2 * hp + 1)):
                ph = 64 * (h % 2)
                bc_full = mm512.tile([128, 512], f32, tag="mm",
                                     name=f"bc_{qt}_{h}")
                bc = bc_full[0:64, :]
                nc.tensor.matmul(bc, lhsT=ones64, rhs=recs[hi],
                                 start=True, stop=True)
                bcs = nrm_pool.tile([64, 512], f32, tag="bcs",
                                    name=f"bcs_{qt}_{h}")
                nc.vector.tensor_copy(out=bcs, in_=bc)
                nc.vector.tensor_mul(
                    out=ctx_sb[ph:ph + 64, hp, 512 * qt:512 * (qt + 1)],
                    in0=cps[0:64, hi, :], in1=bcs)

        pending_norm = None  # (qt, hp, cps, recs) awaiting emission

        for nt in range(4):
            # ---- Q/K projections for the chunk ----
            for wsb, dst, bidx in ((wq_sb, qT_sb, 0), (wk_sb, kT_sb, 1)):
                for dt_ in range(2):
                    ps = mm512.tile([128, 512], f32, tag="mm")
                    for ko in range(8):
                        nc.tensor.matmul(
                            ps,
                            lhsT=wsb[:, ko, 128 * dt_:128 * dt_ + 128],
                            rhs=xT_sb[:, ko, 512 * nt:512 * (nt + 1)],
                            start=(ko == 0),
                            stop=(ko == 7 and not has_qkv_bias),
                        )
                    if has_qkv_bias:
                        nc.tensor.matmul(
                            ps,
                            lhsT=bqkv_sb[:, bidx, 128 * dt_:128 * dt_ + 128],
                            rhs=ones512,
                            start=False, stop=True,
                        )
                    nc.vector.tensor_copy(
                        out=dst[:, dt_, 512 * nt:512 * (nt + 1)], in_=ps)

            # ---- V projection for the chunk (4 x 128-row subtiles) ----
            for sub in range(4):
                n16 = 4 * nt + sub
                psf = mm512.tile([128, 512], f32, tag="mm")
                ps = psf[:, 0:DP]
                for ko in range(8):
                    nc.tensor.matmul(
                        ps,
                        lhsT=xT_sb[:, ko, 128 * n16:128 * n16 + 128],
                        rhs=wv_sb[:, ko],
                        start=(ko == 0),
                        stop=(ko == 7 and not has_qkv_bias),
                    )
                if has_qkv_bias:
                    nc.tensor.matmul(
                        ps,
                        lhsT=ones512[:, 0:128],
                        rhs=bqkv_sb[:, 2, :],
                        start=False, stop=True,
                    )
                nc.vector.tensor_copy(
                    out=v_sb[:, n16, :, 0:DH],
                    in_=ps.rearrange("p (h d) -> p h d", h=HPC))

            # ---- causal attention for q-tile qt = nt ----
            qt = nt
            for hp in range(2):
                heads = (2 * hp, 2 * hp + 1)
                cps = cps_pool.tile([128, 2, 512], f32, tag="ctx",
                                    name=f"cps_{qt}_{hp}")
                n_kt = 4 * qt + 4
                pend = []
                for kt in range(n_kt):
                    j = kt - 4 * qt
                    c0 = 128 * j if j > 0 else 0
                    sp = sps_pool.tile([128, 2, 512], f32, tag="s",
                                       name=f"sp_{qt}_{hp}_{kt}")
                    for hi, h in enumerate(heads):
                        ph = 64 * (h % 2)
                        nc.tensor.matmul(
                            sp[:, hi],
                            lhsT=kT_sb[ph:ph + 64, hp,
                                       128 * kt:128 * kt + 128],
                            rhs=qT_sb[ph:ph + 64, hp,
                                      512 * qt:512 * (qt + 1)],
                            start=True,
                            stop=True,
                        )
                    es = es_pool.tile([128, 2, 512], bf16, tag="es")
                    nc.scalar.activation(
                        out=es[:, :, c0:512],
                        in_=sp[:, :, c0:512],
                        func=AF.Exp, scale=0.125,
                    )
                    if j >= 0:
                        for hi in range(2):
                            nc.vector.tensor_mul(
                                out=es[:, hi, 128 * j:128 * j + 128],
                                in0=es[:, hi, 128 * j:128 * j + 128],
                                in1=tri_sb)
                    pend.append((es, kt, c0))
                    if len(pend) > 2:
                        _emit_av(nc, cps, v_sb, heads, pend.pop(0), n_kt)
                    # the previous pair's normalize rides in here so the
                    # in-order PE never drains at a pair boundary
                    if kt == min(5, n_kt - 1) and pending_norm is not None:
                        emit_normalize(*pending_norm)
                        pending_norm = None
                while pend:
                    _emit_av(nc, cps, v_sb, heads, pend.pop(0), n_kt)
                recs = emit_recip(qt, hp, cps)
                pending_norm = (qt, hp, cps, recs)
            emit_normalize(*pending_norm)
            pending_norm = None

            # ---- partial out-proj for qt: rows x my 256 d' columns ----
            part_sb = part_pool.tile([128, 4, D], bf16, tag="part",
                                     name=f"part_sb_{qt}")
            for ntl in range(4):
                nb = 512 * qt + 128 * ntl
                for Dt in range(2):
                    ps = mm512.tile([128, 512], f32, tag="mm")
                    for hpb in range(2):
                        nc.tensor.matmul(
                            ps,
                            lhsT=ctx_sb[:, hpb, nb:nb + 128],
                            rhs=wo_sb[:, hpb, 512 * Dt:512 * (Dt + 1)],
                            start=(hpb == 0), stop=(hpb == 1),
                        )
                    nc.vector.tensor_copy(
                        out=part_sb[:, ntl, 512 * Dt:512 * (Dt + 1)], in_=ps)
            nc.gpsimd.dma_start(
                part_dram[qt].rearrange("(t p) m -> p t m", p=128), part_sb)
            nc.gpsimd.collective_compute(
                "ReduceScatter", ALU.add,
                replica_groups=GROUPS,
                ins=[part_dram[qt][:, :]],
                outs=[rsout_dram[qt]],
            )

        # ---------- final: readback + residual + LayerNorm ----------
        with tc.tile_pool(name="fin", bufs=4) as fin_pool:
            for qt in range(4):
                rs_sb = fin_pool.tile([128, D], bf16, tag="rs",
                                      name=f"rs_{qt}")
                [nc.sync, nc.scalar][qt % 2].dma_start(rs_sb, rsout_dram[qt])
                yt = fin_pool.tile([128, D], f32, tag="yt", name=f"yt_{qt}")
                nc.vector.tensor_add(out=yt, in0=xres_sb[:, qt], in1=rs_sb)
                st = fin_pool.tile([128, 2, 6], f32, tag="st")
                nc.vector.bn_stats(out=st[:, 0], in_=yt[:, 0:512])
                nc.vector.bn_stats(out=st[:, 1], in_=yt[:, 512:1024])
                mv = fin_pool.tile([128, 2], f32, tag="mv")
                nc.vector.bn_aggr(out=mv, in_=st)
                rstd = fin_pool.tile([128, 1], f32, tag="rstd")
                nc.scalar.activation(out=rstd, in_=mv[:, 1:2], func=AF.Sqrt,
                                     bias=eps_sb, scale=1.0)
                nc.vector.reciprocal(out=rstd, in_=rstd)
                nc.vector.tensor_scalar(
                    out=yt, in0=yt, scalar1=mv[:, 0:1], scalar2=rstd,
                    op0=ALU.subtract, op1=ALU.mult)
                if has_gamma:
                    nc.vector.tensor_mul(out=yt, in0=yt, in1=gamma_sb)
                if has_beta:
                    nc.vector.tensor_add(out=yt, in0=yt, in1=beta_sb)
                nc.sync.dma_start(out[128 * qt:128 * (qt + 1)], yt)

    nc.compile()
    return nc


def _emit_av(nc, cps, v_sb, heads, pend_item, n_kt):
    es, kt, c0 = pend_item
    for hi, h in enumerate(heads):
        nc.tensor.matmul(
            cps[0:65, hi, c0:512],
            lhsT=v_sb[:, kt, h, :],
            rhs=es[:, hi, c0:512],
            start=(kt == 0),
            stop=(kt == n_kt - 1),
        )


def build_nc(flags=(False, False, False)):
    if flags not in _CACHE:
        _CACHE[flags] = _build(flags)
    return _CACHE[flags]


def make_in_maps(inputs):
    import ml_dtypes
    bf = ml_dtypes.bfloat16

    x = np.ascontiguousarray(np.asarray(inputs["x"], dtype=np.float32))
    Wq = np.asarray(inputs["Wq"], np.float32)
    Wk = np.asarray(inputs["Wk"], np.float32)
    Wv = np.asarray(inputs["Wv"], np.float32)
    Wo = np.asarray(inputs["Wo"], np.float32)
    bq = np.asarray(inputs["bq"], np.float32)
    bk = np.asarray(inputs["bk"], np.float32)
    bv = np.asarray(inputs["bv"], np.float32)
    bo = np.asarray(inputs["bo"], np.float32)
    gamma = np.asarray(inputs["ln_gamma"], np.float32)
    beta = np.asarray(inputs["ln_beta"], np.float32)

    has_qkv_bias = bool(np.any(bq) or np.any(bk) or np.any(bv))
    has_gamma = not np.allclose(gamma, 1.0)
    has_beta = bool(np.any(beta))
    flags = (has_qkv_bias, has_gamma, has_beta)

    xres_full = x + bo  # residual with output bias folded in

    in_maps = []
    for c in range(NCORES):
        b, r = c // 4, c % 4
        cols = slice(DP * r, DP * (r + 1))
        # rows this core owns: for each q-tile, its 128-row shard r
        xres_rows = np.concatenate(
            [xres_full[b, 512 * qt + 128 * r:512 * qt + 128 * (r + 1)]
             for qt in range(4)], axis=0)
        m = {
            "xT": np.ascontiguousarray(x[b].T).astype(bf),
            "xres": np.ascontiguousarray(xres_rows),
            "wqT": np.ascontiguousarray(Wq[cols, :].T).astype(bf),
            "wkT": np.ascontiguousarray(Wk[cols, :].T).astype(bf),
            "wvT": np.ascontiguousarray(Wv[cols, :].T).astype(bf),
            "woTm": np.ascontiguousarray(Wo[:, cols].T).astype(bf),
        }
        if has_qkv_bias:
            m["bqkv"] = np.ascontiguousarray(
                np.stack([bq[cols], bk[cols], bv[cols]])[None]).astype(bf)
        if has_gamma:
            m["gamma"] = gamma
        if has_beta:
            m["beta"] = beta
        in_maps.append(m)
    return flags, in_maps


def assemble(results):
    """results: list of per-core dicts with 'out' [512, 1024] (4 qt x 128)."""
    full = np.empty((B, N, D), dtype=np.float32)
    for c in range(NCORES):
        b, r = c // 4, c % 4
        o = results[c]["out"]
        for qt in range(4):
            full[b, 512 * qt + 128 * r:512 * qt + 128 * (r + 1)] = \
                o[128 * qt:128 * (qt + 1)]
    return full


def kernel(**inputs):
    from concourse.bass_utils import run_bass_kernel_spmd

    flags, in_maps = make_in_maps(inputs)
    nc = build_nc(flags)
    res = run_bass_kernel_spmd(nc, in_maps, core_ids=list(range(NCORES)))
    return assemble(res.results)


# revision 14
# speedup vs baseline: 1.0809x; 1.0809x over previous
"""Fused causal-attention block (QKV proj + causal softmax attention + out proj
+ residual + LayerNorm) on 8 Trainium2 NeuronCores.

Sharding: core c -> batch b = c//4, head-group r = c%4 (heads 4r..4r+3,
d' columns 256r..256r+256).  All matmul operands are bf16 (fp32 PSUM
accumulation), which halves DMA traffic and runs the PE at 1 cycle/row.

Single fused pipeline per core, streaming over the four 512-wide n-chunks:
  chunk nt: load xT chunk -> Q/K/V projections for the chunk
            -> causal attention for q-tile qt=nt (flash-style, no max
               subtraction; denominators via an all-ones column in V)
            -> partial output projection (only this core's 256 d' columns
               of Wo) -> stage partial [512,1024] bf16 to DRAM
            -> ReduceScatter(add) over the batch's 4 cores: rank r gets
               rows [128r,128r+128) of the q-tile summed.
Each RS overlaps later chunks' compute; only the last one is exposed.
A final phase reads back the four [128,1024] RS shards, adds the fp32
residual rows, and runs LayerNorm + store.  Host reassembles 8 cores x
4 q-tiles x 128 rows.

Causal masking on diagonal 128x128 blocks: exp(scores) multiplied by an
upper-triangular 0/1 bf16 matrix on the DVE.  The two heads of a pair
compute their K=64 score matmuls at PE row bases 0/64 (disjoint row
groups -> concurrent) into one shared [128,2,512] PSUM tile; one strided
ACT exp call covers both heads.
"""

import numpy as np

B, N, D = 2, 2048, 1024
H, DH = 16, 64
NCORES = 8
HPC = 4          # heads per core
DP = HPC * DH    # 256 d' columns per core
NQ = N // 4      # 512 rows per q-tile (and per core)
LN_EPS = 1e-5
GROUPS = [[0, 1, 2, 3], [4, 5, 6, 7]]

_CACHE = {}


def _build(flags):
    """Build+compile the Bacc program. flags = (has_qkv_bias, has_gamma, has_beta)."""
    import concourse.bass as bass
    import concourse.bacc as bacc
    import concourse.tile as tile
    from concourse import mybir
    from contextlib import ExitStack

    has_qkv_bias, has_gamma, has_beta = flags
    f32 = mybir.dt.float32
    bf16 = mybir.dt.bfloat16
    AF = mybir.ActivationFunctionType
    ALU = mybir.AluOpType

    nc = bacc.Bacc(
        trn_type="TRN2",
        target_bir_lowering=False,
        debug=False,
        num_devices=NCORES,
    )

    xT = nc.dram_tensor("xT", [D, N], bf16, kind="ExternalInput").ap()
    xres = nc.dram_tensor("xres", [NQ, D], f32, kind="ExternalInput").ap()
    wqT = nc.dram_tensor("wqT", [D, DP], bf16, kind="ExternalInput").ap()
    wkT = nc.dram_tensor("wkT", [D, DP], bf16, kind="ExternalInput").ap()
    wvT = nc.dram_tensor("wvT", [D, DP], bf16, kind="ExternalInput").ap()
    woTm = nc.dram_tensor("woTm", [DP, D], bf16, kind="ExternalInput").ap()
    out = nc.dram_tensor("out", [NQ, D], f32, kind="ExternalOutput").ap()
    if has_qkv_bias:
        bqkv = nc.dram_tensor("bqkv", [1, 3, DP], bf16, kind="ExternalInput").ap()
    if has_gamma:
        gamma_d = nc.dram_tensor("gamma", [D], f32, kind="ExternalInput").ap()
    if has_beta:
        beta_d = nc.dram_tensor("beta", [D], f32, kind="ExternalInput").ap()

    # multiplicative causal mask for diagonal blocks: keep k <= q
    # (partition p = k offset, free c = q offset)
    tri_np = np.triu(np.ones((128, 128), np.float32))
    tri_d = nc.inline_tensor(tri_np, name="tri_const").ap()

    with tile.TileContext(nc) as tc, ExitStack() as ctx, \
            nc.allow_low_precision(reason="bf16 operands, fp32 accumulation"):
        singles = ctx.enter_context(tc.tile_pool(name="singles", bufs=1))
        qkv_pool = ctx.enter_context(tc.tile_pool(name="qkv", bufs=1))

        # weights, striped k-on-partitions (one per queue, then x chunks)
        wq_sb = singles.tile([128, 8, DP], bf16, tag="wq")
        wk_sb = singles.tile([128, 8, DP], bf16, tag="wk")
        wv_sb = singles.tile([128, 8, DP], bf16, tag="wv")
        nc.sync.dma_start(wq_sb, wqT.rearrange("(ko p) m -> p ko m", p=128))
        nc.scalar.dma_start(wk_sb, wkT.rearrange("(ko p) m -> p ko m", p=128))
        nc.gpsimd.dma_start(wv_sb, wvT.rearrange("(ko p) m -> p ko m", p=128))

        # prefetch the whole x^T, chunk-major so chunk 0 lands first;
        # every later phase's DMA sits behind these on the three queues.
        xT_sb = qkv_pool.tile([128, 8, N], bf16, tag="xT")
        xT_r = xT.rearrange("(ko p) n -> ko p n", p=128)
        dma_engs = [nc.sync, nc.scalar, nc.gpsimd]
        for nt in range(4):
            for ko in range(8):
                dma_engs[ko % 3].dma_start(
                    xT_sb[:, ko, 512 * nt:512 * (nt + 1)],
                    xT_r[ko][:, 512 * nt:512 * (nt + 1)])

        wo_sb = singles.tile([128, 2, D], bf16, tag="wo")
        nc.scalar.dma_start(wo_sb, woTm.rearrange("(t p) m -> p t m", p=128))

        tri_f = singles.tile([128, 128], f32, tag="tri_f")
        nc.sync.dma_start(tri_f, tri_d)
        tri_sb = singles.tile([128, 128], bf16, tag="tri")
        nc.vector.tensor_copy(out=tri_sb, in_=tri_f)

        ones_f32 = singles.tile([128, 64], f32, tag="ones_f32")
        nc.vector.memset(ones_f32, 1.0)
        ones64 = singles.tile([1, 64], bf16, tag="ones64")
        nc.vector.tensor_copy(out=ones64, in_=ones_f32[0:1, :])
        eps_sb = singles.tile([128, 1], f32, tag="eps")
        nc.vector.memset(eps_sb, LN_EPS)
        if has_qkv_bias:
            o512f = singles.tile([1, 512], f32, tag="o512f")
            nc.vector.memset(o512f, 1.0)
            ones512 = singles.tile([1, 512], bf16, tag="ones512")
            nc.vector.tensor_copy(out=ones512, in_=o512f)
            bqkv_sb = singles.tile([1, 3, DP], bf16, tag="bqkv")
            nc.sync.dma_start(bqkv_sb, bqkv)
        if has_gamma:
            gamma_sb = singles.tile([128, D], f32, tag="gamma")
            nc.sync.dma_start(
                gamma_sb,
                bass.AP(tensor=gamma_d.tensor, offset=gamma_d.offset,
                        ap=[[0, 128]] + gamma_d.ap),
            )
        if has_beta:
            beta_sb = singles.tile([128, D], f32, tag="beta")
            nc.sync.dma_start(
                beta_sb,
                bass.AP(tensor=beta_d.tensor, offset=beta_d.offset,
                        ap=[[0, 128]] + beta_d.ap),
            )

        # persistent activations
        qT_sb = qkv_pool.tile([128, 2, N], bf16, tag="qT")   # Q^T [d'(256), n]
        kT_sb = qkv_pool.tile([128, 2, N], bf16, tag="kT")   # K^T [d'(256), n]
        v_sb = qkv_pool.tile([128, 16, HPC, DH + 1], bf16, tag="v")  # V + ones
        ctx_sb = qkv_pool.tile([128, 2, N], bf16, tag="ctxT")  # normalized ctx^T
        nc.vector.tensor_copy(
            out=v_sb[:, :, :, DH:DH + 1],
            in_=ones_f32.rearrange("p (a b c) -> p a b c", a=16, b=4))

        # residual rows (pure-input DMAs; queued behind the x prefetch)
        xres_sb = singles.tile([128, 4, D], f32, tag="xres")
        for t in range(4):
            [nc.sync, nc.scalar, nc.gpsimd][t % 3].dma_start(
                xres_sb[:, t], xres[128 * t:128 * (t + 1)])

        dram_pool = ctx.enter_context(tc.tile_pool(name="dram", bufs=1,
                                                   space="DRAM"))
        part_dram = [dram_pool.tile([NQ, D], bf16, tag=f"part{qt}",
                                    name=f"part{qt}")
                     for qt in range(4)]
        rsout_dram = [dram_pool.tile([128, D], bf16, tag=f"rsout{qt}",
                                     name=f"rsout{qt}")
                      for qt in range(4)]

        # PSUM pools: mm512 (QKV + out-proj) 2 banks, sp 4 banks, cps 2 banks
        mm512 = ctx.enter_context(tc.tile_pool(name="mm512", bufs=2,
                                               space="PSUM"))
        sps_pool = ctx.enter_context(tc.tile_pool(name="sps", bufs=2,
                                                  space="PSUM"))
        cps_pool = ctx.enter_context(tc.tile_pool(name="cps", bufs=1,
                                                  space="PSUM"))
        es_pool = ctx.enter_context(tc.tile_pool(name="es", bufs=6))
        nrm_pool = ctx.enter_context(tc.tile_pool(name="nrm", bufs=4))
        part_pool = ctx.enter_context(tc.tile_pool(name="part", bufs=2))

        def emit_recip(qt, hp, cps):
            recs = []
            for hi, h in enumerate((2 * hp, 2 * hp + 1)):
                rec = nrm_pool.tile([1, 512], bf16, tag="rec",
                                    name=f"rec_{qt}_{h}")
                nc.vector.reciprocal(out=rec, in_=cps[64:65, hi, :])
                recs.append(rec)
            return recs

        def emit_normalize(qt, hp, cps, recs):
            for hi, h in enumerate((2 * hp, 2 * hp + 1)):
                ph = 64 * (h % 2)
                bc_full = mm512.tile([128, 512], f32, tag="mm",
                                     name=f"bc_{qt}_{h}")
                bc = bc_full[0:64, :]
                nc.tensor.matmul(bc, lhsT=ones64, rhs=recs[hi],
                                 start=True, stop=True)
                bcs = nrm_pool.tile([64, 512], f32, tag="bcs",
                                    name=f"bcs_{qt}_{h}")
                nc.vector.tensor_copy(out=bcs, in_=bc)
                nc.vector.tensor_mul(
                    out=ctx_sb[ph:ph + 64, hp, 512 * qt:512 * (qt + 1)],
                    in0=cps[0:64, hi, :], in1=bcs)

        pending_norm = None  # (qt, hp, cps, recs) awaiting emission

        for nt in range(4):
            # ---- Q/K projections for the chunk ----
            for wsb, dst, bidx in ((wq_sb, qT_sb, 0), (wk_sb, kT_sb, 1)):
                for dt_ in range(2):
                    ps = mm512.tile([128, 512], f32, tag="mm")
                    for ko in range(8):
                        nc.tensor.matmul(
                            ps,
                            lhsT=wsb[:, ko, 128 * dt_:128 * dt_ + 128],
                            rhs=xT_sb[:, ko, 512 * nt:512 * (nt + 1)],
                            start=(ko == 0),
                            stop=(ko == 7 and not has_qkv_bias),
                        )
                    if has_qkv_bias:
                        nc.tensor.matmul(
                            ps,
                            lhsT=bqkv_sb[:, bidx, 128 * dt_:128 * dt_ + 128],
                            rhs=ones512,
                            start=False, stop=True,
                        )
                    nc.vector.tensor_copy(
                        out=dst[:, dt_, 512 * nt:512 * (nt + 1)], in_=ps)

            # ---- V projection for the chunk (4 x 128-row subtiles) ----
            for sub in range(4):
                n16 = 4 * nt + sub
                psf = mm512.tile([128, 512], f32, tag="mm")
                ps = psf[:, 0:DP]
                for ko in range(8):
                    nc.tensor.matmul(
                        ps,
                        lhsT=xT_sb[:, ko, 128 * n16:128 * n16 + 128],
                        rhs=wv_sb[:, ko],
                        start=(ko == 0),
                        stop=(ko == 7 and not has_qkv_bias),
                    )
                if has_qkv_bias:
                    nc.tensor.matmul(
                        ps,
                        lhsT=ones512[:, 0:128],
                        rhs=bqkv_sb[:, 2, :],
                        start=False, stop=True,
                    )
                nc.vector.tensor_copy(
                    out=v_sb[:, n16, :, 0:DH],
                    in_=ps.rearrange("p (h d) -> p h d", h=HPC))

            # ---- causal attention for q-tile qt = nt ----
            qt = nt
            for hp in range(2):
                heads = (2 * hp, 2 * hp + 1)
                cps = cps_pool.tile([128, 2, 512], f32, tag="ctx",
                                    name=f"cps_{qt}_{hp}")
                n_kt = 4 * qt + 4
                pend = []
                for kt in range(n_kt):
                    j = kt - 4 * qt
                    c0 = 128 * j if j > 0 else 0
                    sp = sps_pool.tile([128, 2, 512], f32, tag="s",
                                       name=f"sp_{qt}_{hp}_{kt}")
                    for hi, h in enumerate(heads):
                        ph = 64 * (h % 2)
                        nc.tensor.matmul(
                            sp[:, hi],
                            lhsT=kT_sb[ph:ph + 64, hp,
                                       128 * kt:128 * kt + 128],
                            rhs=qT_sb[ph:ph + 64, hp,
                                      512 * qt:512 * (qt + 1)],
                            start=True,
                            stop=True,
                        )
                    es = es_pool.tile([128, 2, 512], bf16, tag="es")
                    nc.scalar.activation(
                        out=es[:, :, c0:512],
                        in_=sp[:, :, c0:512],
                        func=AF.Exp, scale=0.125,
                    )
                    if j >= 0:
                        for hi in range(2):
                            nc.vector.tensor_mul(
                                out=es[:, hi, 128 * j:128 * j + 128],
                                in0=es[:, hi, 128 * j:128 * j + 128],
                                in1=tri_sb)
                    pend.append((es, kt, c0))
                    if len(pend) > 2:
                        _emit_av(nc, cps, v_sb, heads, pend.pop(0), n_kt)
                    # the previous pair's normalize rides in here so the
                    # in-order PE never drains at a pair boundary
                    if kt == min(5, n_kt - 1) and pending_norm is not None:
                        emit_normalize(*pending_norm)
                        pending_norm = None
                while pend:
                    _emit_av(nc, cps, v_sb, heads, pend.pop(0), n_kt)
                recs = emit_recip(qt, hp, cps)
                pending_norm = (qt, hp, cps, recs)
            emit_normalize(*pending_norm)
            pending_norm = None

            # ---- partial out-proj for qt: rows x my 256 d' columns ----
            part_sb = part_pool.tile([128, 4, D], bf16, tag="part",
                                     name=f"part_sb_{qt}")
            for ntl in range(4):
                nb = 512 * qt + 128 * ntl
                for Dt in range(2):
                    ps = mm512.tile([128, 512], f32, tag="mm")
                    for hpb in range(2):
                        nc.tensor.matmul(
                            ps,
                            lhsT=ctx_sb[:, hpb, nb:nb + 128],
                            rhs=wo_sb[:, hpb, 512 * Dt:512 * (Dt + 1)],
                            start=(hpb == 0), stop=(hpb == 1),
                        )
                    nc.vector.tensor_copy(
                        out=part_sb[:, ntl, 512 * Dt:512 * (Dt + 1)], in_=ps)
            nc.gpsimd.dma_start(
                part_dram[qt].rearrange("(t p) m -> p t m", p=128), part_sb)
            nc.gpsimd.collective_compute(
                "ReduceScatter", ALU.add,
                replica_groups=GROUPS,
                ins=[part_dram[qt][:, :]],
                outs=[rsout_dram[qt]],
            )

        # ---------- final: readback + residual + LayerNorm ----------
        # tile_wait_until keeps the scheduler from hoisting this RS-gated
        # block into the middle of the engine queues (its cost model thinks
        # collectives are fast; in reality each RS is ~20us, and a hoisted
        # LN stalls every attention op queued behind it).
        with tc.tile_pool(name="fin", bufs=4) as fin_pool, \
                tc.tile_wait_until(1.0):
            for qt in range(4):
                rs_sb = fin_pool.tile([128, D], bf16, tag="rs",
                                      name=f"rs_{qt}")
                [nc.sync, nc.scalar][qt % 2].dma_start(rs_sb, rsout_dram[qt])
                yt = fin_pool.tile([128, D], f32, tag="yt", name=f"yt_{qt}")
                nc.vector.tensor_add(out=yt, in0=xres_sb[:, qt], in1=rs_sb)
                st = fin_pool.tile([128, 2, 6], f32, tag="st")
                nc.vector.bn_stats(out=st[:, 0], in_=yt[:, 0:512])
                nc.vector.bn_stats(out=st[:, 1], in_=yt[:, 512:1024])
                mv = fin_pool.tile([128, 2], f32, tag="mv")
                nc.vector.bn_aggr(out=mv, in_=st)
                rstd = fin_pool.tile([128, 1], f32, tag="rstd")
                nc.scalar.activation(out=rstd, in_=mv[:, 1:2], func=AF.Sqrt,
                                     bias=eps_sb, scale=1.0)
                nc.vector.reciprocal(out=rstd, in_=rstd)
                nc.vector.tensor_scalar(
                    out=yt, in0=yt, scalar1=mv[:, 0:1], scalar2=rstd,
                    op0=ALU.subtract, op1=ALU.mult)
                if has_gamma:
                    nc.vector.tensor_mul(out=yt, in0=yt, in1=gamma_sb)
                if has_beta:
                    nc.vector.tensor_add(out=yt, in0=yt, in1=beta_sb)
                nc.sync.dma_start(out[128 * qt:128 * (qt + 1)], yt)

    nc.compile()
    return nc


def _emit_av(nc, cps, v_sb, heads, pend_item, n_kt):
    es, kt, c0 = pend_item
    for hi, h in enumerate(heads):
        nc.tensor.matmul(
            cps[0:65, hi, c0:512],
            lhsT=v_sb[:, kt, h, :],
            rhs=es[:, hi, c0:512],
            start=(kt == 0),
            stop=(kt == n_kt - 1),
        )


def build_nc(flags=(False, False, False)):
    if flags not in _CACHE:
        _CACHE[flags] = _build(flags)
    return _CACHE[flags]


def make_in_maps(inputs):
    import ml_dtypes
    bf = ml_dtypes.bfloat16

    x = np.ascontiguousarray(np.asarray(inputs["x"], dtype=np.float32))
    Wq = np.asarray(inputs["Wq"], np.float32)
    Wk = np.asarray(inputs["Wk"], np.float32)
    Wv = np.asarray(inputs["Wv"], np.float32)
    Wo = np.asarray(inputs["Wo"], np.float32)
    bq = np.asarray(inputs["bq"], np.float32)
    bk = np.asarray(inputs["bk"], np.float32)
    bv = np.asarray(inputs["bv"], np.float32)
    bo = np.asarray(inputs["bo"], np.float32)
    gamma = np.asarray(inputs["ln_gamma"], np.float32)
    beta = np.asarray(inputs["ln_beta"], np.float32)

    has_qkv_bias = bool(np.any(bq) or np.any(bk) or np.any(bv))
    has_gamma = not np.allclose(gamma, 1.0)
    has_beta = bool(np.any(beta))
    flags = (has_qkv_bias, has_gamma, has_beta)

    xres_full = x + bo  # residual with output bias folded in

    in_maps = []
    for c in range(NCORES):
        b, r = c // 4, c % 4
        cols = slice(DP * r, DP * (r + 1))
        # rows this core owns: for each q-tile, its 128-row shard r
        xres_rows = np.concatenate(
            [xres_full[b, 512 * qt + 128 * r:512 * qt + 128 * (r + 1)]
             for qt in range(4)], axis=0)
        m = {
            "xT": np.ascontiguousarray(x[b].T).astype(bf),
            "xres": np.ascontiguousarray(xres_rows),
            "wqT": np.ascontiguousarray(Wq[cols, :].T).astype(bf),
            "wkT": np.ascontiguousarray(Wk[cols, :].T).astype(bf),
            "wvT": np.ascontiguousarray(Wv[cols, :].T).astype(bf),
            "woTm": np.ascontiguousarray(Wo[:, cols].T).astype(bf),
        }
        if has_qkv_bias:
            m["bqkv"] = np.ascontiguousarray(
                np.stack([bq[cols], bk[cols], bv[cols]])[None]).astype(bf)
        if has_gamma:
            m["gamma"] = gamma
        if has_beta:
            m["beta"] = beta
        in_maps.append(m)
    return flags, in_maps


def assemble(results):
    """results: list of per-core dicts with 'out' [512, 1024] (4 qt x 128)."""
    full = np.empty((B, N, D), dtype=np.float32)
    for c in range(NCORES):
        b, r = c // 4, c % 4
        o = results[c]["out"]
        for qt in range(4):
            full[b, 512 * qt + 128 * r:512 * qt + 128 * (r + 1)] = \
                o[128 * qt:128 * (qt + 1)]
    return full


def kernel(**inputs):
    from concourse.bass_utils import run_bass_kernel_spmd

    flags, in_maps = make_in_maps(inputs)
    nc = build_nc(flags)
    res = run_bass_kernel_spmd(nc, in_maps, core_ids=list(range(NCORES)))
    return assemble(res.results)


# revision 23
# speedup vs baseline: 1.1573x; 1.0707x over previous
"""Fused causal-attention block (QKV proj + causal softmax attention + out proj
+ residual + LayerNorm) on 8 Trainium2 NeuronCores.

Sharding: core c -> batch b = c//4, head-group r = c%4 (heads 4r..4r+3,
d' columns 256r..256r+256).  All matmul operands are bf16 (fp32 PSUM
accumulation), which halves DMA traffic and runs the PE at 1 cycle/row.

Single fused pipeline per core, streaming over the four 512-wide n-chunks:
  chunk nt: load xT chunk -> Q/K/V projections for the chunk
            -> causal attention for q-tile qt=nt (flash-style, no max
               subtraction; denominators via an all-ones column in V)
            -> partial output projection (only this core's 256 d' columns
               of Wo) -> stage partial [512,1024] bf16 to DRAM
            -> ReduceScatter(add) over the batch's 4 cores: rank r gets
               rows [128r,128r+128) of the q-tile summed.
Each RS overlaps later chunks' compute; only the last one is exposed.
A final phase reads back the four [128,1024] RS shards, adds the fp32
residual rows, and runs LayerNorm + store.  Host reassembles 8 cores x
4 q-tiles x 128 rows.

Causal masking on diagonal 128x128 blocks: exp(scores) multiplied by an
upper-triangular 0/1 bf16 matrix on the DVE.  The two heads of a pair
compute their K=64 score matmuls at PE row bases 0/64 (disjoint row
groups -> concurrent) into one shared [128,2,512] PSUM tile; one strided
ACT exp call covers both heads.
"""

import numpy as np

B, N, D = 2, 2048, 1024
H, DH = 16, 64
NCORES = 8
HPC = 4          # heads per core
DP = HPC * DH    # 256 d' columns per core
NQ = N // 4      # 512 rows per q-tile (and per core)
LN_EPS = 1e-5
GROUPS = [[0, 1, 2, 3], [4, 5, 6, 7]]

_CACHE = {}


def _build(flags):
    """Build+compile the Bacc program. flags = (has_qkv_bias, has_gamma, has_beta)."""
    import concourse.bass as bass
    import concourse.bacc as bacc
    import concourse.tile as tile
    from concourse import mybir
    from contextlib import ExitStack

    has_qkv_bias, has_gamma, has_beta = flags
    f32 = mybir.dt.float32
    bf16 = mybir.dt.bfloat16
    AF = mybir.ActivationFunctionType
    ALU = mybir.AluOpType

    nc = bacc.Bacc(
        trn_type="TRN2",
        target_bir_lowering=False,
        debug=False,
        num_devices=NCORES,
    )

    xT = nc.dram_tensor("xT", [D, N], bf16, kind="ExternalInput").ap()
    xres = nc.dram_tensor("xres", [NQ, D], f32, kind="ExternalInput").ap()
    wqT = nc.dram_tensor("wqT", [D, DP], bf16, kind="ExternalInput").ap()
    wkT = nc.dram_tensor("wkT", [D, DP], bf16, kind="ExternalInput").ap()
    wvT = nc.dram_tensor("wvT", [D, DP], bf16, kind="ExternalInput").ap()
    woTm = nc.dram_tensor("woTm", [DP, D], bf16, kind="ExternalInput").ap()
    out = nc.dram_tensor("out", [NQ, D], f32, kind="ExternalOutput").ap()
    if has_qkv_bias:
        bqkv = nc.dram_tensor("bqkv", [1, 3, DP], bf16, kind="ExternalInput").ap()
    if has_gamma:
        gamma_d = nc.dram_tensor("gamma", [D], f32, kind="ExternalInput").ap()
    if has_beta:
        beta_d = nc.dram_tensor("beta", [D], f32, kind="ExternalInput").ap()

    # multiplicative causal mask for diagonal blocks: keep k <= q
    # (partition p = k offset, free c = q offset)
    tri_np = np.triu(np.ones((128, 128), np.float32))
    tri_d = nc.inline_tensor(tri_np, name="tri_const").ap()

    with tile.TileContext(nc) as tc, ExitStack() as ctx, \
            nc.allow_low_precision(reason="bf16 operands, fp32 accumulation"):
        singles = ctx.enter_context(tc.tile_pool(name="singles", bufs=1))
        qkv_pool = ctx.enter_context(tc.tile_pool(name="qkv", bufs=1))

        # weights, striped k-on-partitions (one per queue, then x chunks)
        wq_sb = singles.tile([128, 8, DP], bf16, tag="wq")
        wk_sb = singles.tile([128, 8, DP], bf16, tag="wk")
        wv_sb = singles.tile([128, 8, DP], bf16, tag="wv")
        nc.sync.dma_start(wq_sb, wqT.rearrange("(ko p) m -> p ko m", p=128))
        nc.scalar.dma_start(wk_sb, wkT.rearrange("(ko p) m -> p ko m", p=128))
        nc.gpsimd.dma_start(wv_sb, wvT.rearrange("(ko p) m -> p ko m", p=128))

        # prefetch the whole x^T, chunk-major so chunk 0 lands first.
        # chunk 0 rides the otherwise-idle tensor/vector queues so the
        # first Q matmul can start ~5us in; later chunks go behind the
        # weights on the three DMA-dedicated queues.
        xT_sb = qkv_pool.tile([128, 8, N], bf16, tag="xT")
        xT_r = xT.rearrange("(ko p) n -> ko p n", p=128)
        dma_engs = [nc.sync, nc.scalar, nc.gpsimd]
        for ko in range(8):
            dma_engs[ko % 3].dma_start(
                xT_sb[:, ko, 0:512], xT_r[ko][:, 0:512])
        # chunks 1-3 in one DMA per ko: 3KB-contiguous partition lines
        for ko in range(8):
            dma_engs[ko % 3].dma_start(
                xT_sb[:, ko, 512:2048], xT_r[ko][:, 512:2048])

        wo_sb = singles.tile([128, 2, D], bf16, tag="wo")
        nc.scalar.dma_start(wo_sb, woTm.rearrange("(t p) m -> p t m", p=128))

        tri_f = singles.tile([128, 128], f32, tag="tri_f")
        nc.sync.dma_start(tri_f, tri_d)
        tri_sb = singles.tile([128, 128], bf16, tag="tri")
        nc.vector.tensor_copy(out=tri_sb, in_=tri_f)

        ones_f32 = singles.tile([128, 64], f32, tag="ones_f32")
        nc.vector.memset(ones_f32, 1.0)
        ones64 = singles.tile([1, 64], bf16, tag="ones64")
        nc.vector.tensor_copy(out=ones64, in_=ones_f32[0:1, :])
        eps_sb = singles.tile([128, 1], f32, tag="eps")
        nc.vector.memset(eps_sb, LN_EPS)
        if has_qkv_bias:
            o512f = singles.tile([1, 512], f32, tag="o512f")
            nc.vector.memset(o512f, 1.0)
            ones512 = singles.tile([1, 512], bf16, tag="ones512")
            nc.vector.tensor_copy(out=ones512, in_=o512f)
            bqkv_sb = singles.tile([1, 3, DP], bf16, tag="bqkv")
            nc.sync.dma_start(bqkv_sb, bqkv)
        if has_gamma:
            gamma_sb = singles.tile([128, D], f32, tag="gamma")
            nc.sync.dma_start(
                gamma_sb,
                bass.AP(tensor=gamma_d.tensor, offset=gamma_d.offset,
                        ap=[[0, 128]] + gamma_d.ap),
            )
        if has_beta:
            beta_sb = singles.tile([128, D], f32, tag="beta")
            nc.sync.dma_start(
                beta_sb,
                bass.AP(tensor=beta_d.tensor, offset=beta_d.offset,
                        ap=[[0, 128]] + beta_d.ap),
            )

        # persistent activations
        qT_sb = qkv_pool.tile([128, 2, N], bf16, tag="qT")   # Q^T [d'(256), n]
        kT_sb = qkv_pool.tile([128, 2, N], bf16, tag="kT")   # K^T [d'(256), n]
        v_sb = qkv_pool.tile([128, 16, HPC, DH + 1], bf16, tag="v")  # V + ones
        ctx_sb = qkv_pool.tile([128, 2, N], bf16, tag="ctxT")  # normalized ctx^T
        nc.vector.tensor_copy(
            out=v_sb[:, :, :, DH:DH + 1],
            in_=ones_f32.rearrange("p (a b c) -> p a b c", a=16, b=4))

        # residual rows (pure-input DMAs; queued behind the x prefetch)
        xres_sb = singles.tile([128, 4, D], f32, tag="xres")
        for t in range(4):
            [nc.sync, nc.scalar, nc.gpsimd][t % 3].dma_start(
                xres_sb[:, t], xres[128 * t:128 * (t + 1)])

        dram_pool = ctx.enter_context(tc.tile_pool(name="dram", bufs=1,
                                                   space="DRAM"))
        part_dram = [dram_pool.tile([NQ, D], bf16, tag=f"part{qt}",
                                    name=f"part{qt}")
                     for qt in range(4)]
        rsout_dram = [dram_pool.tile([128, D], bf16, tag=f"rsout{qt}",
                                     name=f"rsout{qt}")
                      for qt in range(4)]

        # PSUM pools (8 banks): mm512 (QKV + out-proj + bc) 2, scores (bf16,
        # 1 bank per buf) 2, AV accumulators 4.
        mm512 = ctx.enter_context(tc.tile_pool(name="mm512", bufs=2,
                                               space="PSUM"))
        sps_pool = ctx.enter_context(tc.tile_pool(name="sps", bufs=2,
                                                  space="PSUM"))
        cps_pool = ctx.enter_context(tc.tile_pool(name="cps", bufs=1,
                                                  space="PSUM"))
        es_pool = ctx.enter_context(tc.tile_pool(name="es", bufs=6))
        nrm_pool = ctx.enter_context(tc.tile_pool(name="nrm", bufs=4))
        part_pool = ctx.enter_context(tc.tile_pool(name="part", bufs=2))

        def emit_recip(qt, hp, cps):
            recs = []
            for hi, h in enumerate((2 * hp, 2 * hp + 1)):
                # bounce the denominator row to SBUF, then the ~5x-faster
                # approximate reciprocal (18 bits ~ plenty for softmax)
                den = nrm_pool.tile([1, 512], f32, tag="den",
                                    name=f"den_{qt}_{h}")
                nc.vector.tensor_copy(out=den, in_=cps[64:65, hi, :])
                recf = nrm_pool.tile([1, 512], f32, tag="recf",
                                     name=f"recf_{qt}_{h}")
                nc.vector.reciprocal_approx_fast(out=recf, in_=den)
                rec = nrm_pool.tile([1, 512], bf16, tag="rec",
                                    name=f"rec_{qt}_{h}")
                nc.vector.tensor_copy(out=rec, in_=recf)
                recs.append(rec)
            return recs

        def emit_normalize(qt, hp, cps, recs):
            for hi, h in enumerate((2 * hp, 2 * hp + 1)):
                ph = 64 * (h % 2)
                bc_full = mm512.tile([128, 512], f32, tag="mm",
                                     name=f"bc_{qt}_{h}")
                bc = bc_full[0:64, :]
                nc.tensor.matmul(bc, lhsT=ones64, rhs=recs[hi],
                                 start=True, stop=True)
                bcs = nrm_pool.tile([64, 512], f32, tag="bcs",
                                    name=f"bcs_{qt}_{h}")
                nc.vector.tensor_copy(out=bcs, in_=bc)
                nc.vector.tensor_mul(
                    out=ctx_sb[ph:ph + 64, hp, 512 * qt:512 * (qt + 1)],
                    in0=cps[0:64, hi, :], in1=bcs)

        pending_norm = None  # (qt, hp, cps, recs) awaiting emission

        for nt in range(4):
            # ---- Q/K projections for the chunk ----
            for wsb, dst, bidx in ((wq_sb, qT_sb, 0), (wk_sb, kT_sb, 1)):
                for dt_ in range(2):
                    ps = mm512.tile([128, 512], f32, tag="mm")
                    for ko in range(8):
                        nc.tensor.matmul(
                            ps,
                            lhsT=wsb[:, ko, 128 * dt_:128 * dt_ + 128],
                            rhs=xT_sb[:, ko, 512 * nt:512 * (nt + 1)],
                            start=(ko == 0),
                            stop=(ko == 7 and not has_qkv_bias),
                        )
                    if has_qkv_bias:
                        nc.tensor.matmul(
                            ps,
                            lhsT=bqkv_sb[:, bidx, 128 * dt_:128 * dt_ + 128],
                            rhs=ones512,
                            start=False, stop=True,
                        )
                    nc.vector.tensor_copy(
                        out=dst[:, dt_, 512 * nt:512 * (nt + 1)], in_=ps)

            # ---- V projection for the chunk (4 x 128-row subtiles) ----
            for sub in range(4):
                n16 = 4 * nt + sub
                psf = mm512.tile([128, 512], f32, tag="mm")
                ps = psf[:, 0:DP]
                for ko in range(8):
                    nc.tensor.matmul(
                        ps,
                        lhsT=xT_sb[:, ko, 128 * n16:128 * n16 + 128],
                        rhs=wv_sb[:, ko],
                        start=(ko == 0),
                        stop=(ko == 7 and not has_qkv_bias),
                    )
                if has_qkv_bias:
                    nc.tensor.matmul(
                        ps,
                        lhsT=ones512[:, 0:128],
                        rhs=bqkv_sb[:, 2, :],
                        start=False, stop=True,
                    )
                nc.vector.tensor_copy(
                    out=v_sb[:, n16, :, 0:DH],
                    in_=ps.rearrange("p (h d) -> p h d", h=HPC))

            # ---- causal attention for q-tile qt = nt ----
            qt = nt
            for hp in range(2):
                heads = (2 * hp, 2 * hp + 1)
                cps = cps_pool.tile([128, 2, 512], f32, tag="ctx",
                                    name=f"cps_{qt}_{hp}")
                n_kt = 4 * qt + 4
                pend = []
                for kt in range(n_kt):
                    j = kt - 4 * qt
                    c0 = 128 * j if j > 0 else 0
                    sp = sps_pool.tile([128, 2, 512], f32, tag="s",
                                       name=f"sp_{qt}_{hp}_{kt}")
                    for hi, h in enumerate(heads):
                        ph = 64 * (h % 2)
                        nc.tensor.matmul(
                            sp[:, hi],
                            lhsT=kT_sb[ph:ph + 64, hp,
                                       128 * kt:128 * kt + 128],
                            rhs=qT_sb[ph:ph + 64, hp,
                                      512 * qt:512 * (qt + 1)],
                            start=True,
                            stop=True,
                        )
                    es = es_pool.tile([128, 2, 512], bf16, tag="es")
                    nc.scalar.activation(
                        out=es[:, :, c0:512],
                        in_=sp[:, :, c0:512],
                        func=AF.Exp, scale=0.125,
                    )
                    if j >= 0:
                        for hi in range(2):
                            nc.vector.tensor_mul(
                                out=es[:, hi, 128 * j:128 * j + 128],
                                in0=es[:, hi, 128 * j:128 * j + 128],
                                in1=tri_sb)
                    pend.append((es, kt, c0))
                    if len(pend) > 2:
                        _emit_av(nc, cps, v_sb, heads, pend.pop(0), n_kt)
                    # the previous pair's normalize rides in here so the
                    # in-order PE never drains at a pair boundary
                    if kt == min(5, n_kt - 1) and pending_norm is not None:
                        emit_normalize(*pending_norm)
                        pending_norm = None
                while pend:
                    _emit_av(nc, cps, v_sb, heads, pend.pop(0), n_kt)
                recs = emit_recip(qt, hp, cps)
                pending_norm = (qt, hp, cps, recs)
            emit_normalize(*pending_norm)
            pending_norm = None

            # ---- partial out-proj for qt: rows x my 256 d' columns ----
            part_sb = part_pool.tile([128, 4, D], bf16, tag="part",
                                     name=f"part_sb_{qt}")
            for ntl in range(4):
                nb = 512 * qt + 128 * ntl
                for Dt in range(2):
                    ps = mm512.tile([128, 512], f32, tag="mm")
                    for hpb in range(2):
                        nc.tensor.matmul(
                            ps,
                            lhsT=ctx_sb[:, hpb, nb:nb + 128],
                            rhs=wo_sb[:, hpb, 512 * Dt:512 * (Dt + 1)],
                            start=(hpb == 0), stop=(hpb == 1),
                        )
                    nc.vector.tensor_copy(
                        out=part_sb[:, ntl, 512 * Dt:512 * (Dt + 1)], in_=ps)
            nc.gpsimd.dma_start(
                part_dram[qt].rearrange("(t p) m -> p t m", p=128), part_sb)
            nc.gpsimd.collective_compute(
                "ReduceScatter", ALU.add,
                replica_groups=GROUPS,
                ins=[part_dram[qt][:, :]],
                outs=[rsout_dram[qt]],
            )

        # ---------- final: readback + residual + LayerNorm ----------
        # tile_wait_until keeps the scheduler from hoisting this RS-gated
        # block into the middle of the engine queues (its cost model thinks
        # collectives are fast; in reality each RS is ~20us, and a hoisted
        # LN stalls every attention op queued behind it).
        with tc.tile_pool(name="fin", bufs=4) as fin_pool, \
                tc.tile_wait_until(1.0):
            for qt in range(4):
                rs_sb = fin_pool.tile([128, D], bf16, tag="rs",
                                      name=f"rs_{qt}")
                # all final DMAs ride the sync queue in per-qt order so an
                # RS-gated readback never blocks another qt's LN math
                nc.sync.dma_start(rs_sb, rsout_dram[qt])
                yt = fin_pool.tile([128, D], f32, tag="yt", name=f"yt_{qt}")
                nc.vector.tensor_add(out=yt, in0=xres_sb[:, qt], in1=rs_sb)
                st = fin_pool.tile([128, 2, 6], f32, tag="st")
                nc.vector.bn_stats(out=st[:, 0], in_=yt[:, 0:512])
                nc.vector.bn_stats(out=st[:, 1], in_=yt[:, 512:1024])
                mv = fin_pool.tile([128, 2], f32, tag="mv")
                nc.vector.bn_aggr(out=mv, in_=st)
                rstd = fin_pool.tile([128, 1], f32, tag="rstd")
                nc.scalar.activation(out=rstd, in_=mv[:, 1:2], func=AF.Sqrt,
                                     bias=eps_sb, scale=1.0)
                nc.vector.reciprocal(out=rstd, in_=rstd)
                nc.vector.tensor_scalar(
                    out=yt, in0=yt, scalar1=mv[:, 0:1], scalar2=rstd,
                    op0=ALU.subtract, op1=ALU.mult)
                if has_gamma:
                    nc.vector.tensor_mul(out=yt, in0=yt, in1=gamma_sb)
                if has_beta:
                    nc.vector.tensor_add(out=yt, in0=yt, in1=beta_sb)
                nc.sync.dma_start(out[128 * qt:128 * (qt + 1)], yt)

    nc.compile()
    return nc


def _emit_av(nc, cps, v_sb, heads, pend_item, n_kt):
    es, kt, c0 = pend_item
    for hi, h in enumerate(heads):
        nc.tensor.matmul(
            cps[0:65, hi, c0:512],
            lhsT=v_sb[:, kt, h, :],
            rhs=es[:, hi, c0:512],
            start=(kt == 0),
            stop=(kt == n_kt - 1),
        )


def build_nc(flags=(False, False, False)):
    if flags not in _CACHE:
        _CACHE[flags] = _build(flags)
    return _CACHE[flags]


def make_in_maps(inputs):
    import ml_dtypes
    bf = ml_dtypes.bfloat16

    x = np.ascontiguousarray(np.asarray(inputs["x"], dtype=np.float32))
    Wq = np.asarray(inputs["Wq"], np.float32)
    Wk = np.asarray(inputs["Wk"], np.float32)
    Wv = np.asarray(inputs["Wv"], np.float32)
    Wo = np.asarray(inputs["Wo"], np.float32)
    bq = np.asarray(inputs["bq"], np.float32)
    bk = np.asarray(inputs["bk"], np.float32)
    bv = np.asarray(inputs["bv"], np.float32)
    bo = np.asarray(inputs["bo"], np.float32)
    gamma = np.asarray(inputs["ln_gamma"], np.float32)
    beta = np.asarray(inputs["ln_beta"], np.float32)

    has_qkv_bias = bool(np.any(bq) or np.any(bk) or np.any(bv))
    has_gamma = not np.allclose(gamma, 1.0)
    has_beta = bool(np.any(beta))
    flags = (has_qkv_bias, has_gamma, has_beta)

    xres_full = x + bo  # residual with output bias folded in

    in_maps = []
    for c in range(NCORES):
        b, r = c // 4, c % 4
        cols = slice(DP * r, DP * (r + 1))
        # rows this core owns: for each q-tile, its 128-row shard r
        xres_rows = np.concatenate(
            [xres_full[b, 512 * qt + 128 * r:512 * qt + 128 * (r + 1)]
             for qt in range(4)], axis=0)
        m = {
            "xT": np.ascontiguousarray(x[b].T).astype(bf),
            "xres": np.ascontiguousarray(xres_rows),
            "wqT": np.ascontiguousarray(Wq[cols, :].T).astype(bf),
            "wkT": np.ascontiguousarray(Wk[cols, :].T).astype(bf),
            "wvT": np.ascontiguousarray(Wv[cols, :].T).astype(bf),
            "woTm": np.ascontiguousarray(Wo[:, cols].T).astype(bf),
        }
        if has_qkv_bias:
            m["bqkv"] = np.ascontiguousarray(
                np.stack([bq[cols], bk[cols], bv[cols]])[None]).astype(bf)
        if has_gamma:
            m["gamma"] = gamma
        if has_beta:
            m["beta"] = beta
        in_maps.append(m)
    return flags, in_maps


def assemble(results):
    """results: list of per-core dicts with 'out' [512, 1024] (4 qt x 128)."""
    full = np.empty((B, N, D), dtype=np.float32)
    for c in range(NCORES):
        b, r = c // 4, c % 4
        o = results[c]["out"]
        for qt in range(4):
            full[b, 512 * qt + 128 * r:512 * qt + 128 * (r + 1)] = \
                o[128 * qt:128 * (qt + 1)]
    return full


def kernel(**inputs):
    from concourse.bass_utils import run_bass_kernel_spmd

    flags, in_maps = make_in_maps(inputs)
    nc = build_nc(flags)
    res = run_bass_kernel_spmd(nc, in_maps, core_ids=list(range(NCORES)))
    return assemble(res.results)
